# revision 28
# baseline (speedup 1.0000x reference)
"""HetGAT Trainium2 kernel: 8-core dst-sharded Bass/Tile implementation.

Per core (SPMD over 8 NeuronCores): host folds relation matrices into
projection weights, sorts edges by dst, pads per-dst-tile edge groups, builds
one-hot scatter blocks + wrapped int16 gather indices. Device: fp16 PE
projections -> kr|vr and q-tilde DRAM tables; per dst tile: dma_gather rows,
DVE dot + ACT exp (exact softmax without max subtraction, |logit| < 4),
one-hot matmul scatter into PSUM; gelu -> alw -> elu; AllGather a1 between
layers; lin1 -> BN (AllReduce of masked stats) -> lin2 -> normalize/softmax.
"""
import numpy as np
from contextlib import ExitStack

import concourse.bass as bass
import concourse.tile as tile
from concourse import bacc, mybir
from concourse.bass_utils import run_bass_kernel_spmd
from concourse.masks import make_identity

N = 20000
E = 400000
H = 10
P = 128
NCORES = 8
TPC = 20
SLICE = TPC * P           # 2560
NTOT = NCORES * SLICE     # 20480
NT_N = (N + P - 1) // P   # 157
NT_A = NTOT // P          # 160

F1, D1, HD1 = 128, 16, 160
F2, D2, HD2 = 160, 32, 320
KV1, KV2 = 2 * HD1, 2 * HD2
KVP1, KVP2 = 384, 640     # padded gather row widths (f16 elems; bytes % 256 == 0)
QP1, QP2 = 256, 384
LIN = 512
FA, FU = 128, 129

f32 = mybir.dt.float32
f16 = mybir.dt.float16
i16 = mybir.dt.int16
i8 = mybir.dt.int8
AF = mybir.ActivationFunctionType
OP = mybir.AluOpType
AX = mybir.AxisListType

_cache = {}


# ============================ host prep ============================
def _blockdiag(mats):
    Hh, Da, Db = mats.shape
    out = np.zeros((Hh * Da, Hh * Db), np.float32)
    for h in range(Hh):
        out[h * Da:(h + 1) * Da, h * Db:(h + 1) * Db] = mats[h]
    return out


def _fold_weights(inp, pref, D):
    g = lambda k: np.asarray(inp[pref + k], np.float32)
    kw, kb, qw, qb = g('kw'), g('kb'), g('qw'), g('qb')
    vw, vb = g('vw'), g('vb')
    arel, mrel, prel = g('arel'), g('mrel'), g('prel')
    scale = 1.0 / np.sqrt(D)
    wkv, wq = [], []
    for r in (0, 1):
        s, d = (0, 1) if r == 0 else (1, 0)
        A, M = _blockdiag(arel[r]), _blockdiag(mrel[r])
        wkv.append(np.vstack([np.hstack([kw[s] @ A, vw[s] @ M]),
                              np.hstack([kb[s] @ A, vb[s] @ M])[None, :]]
                             ).astype(np.float16))
        pr = np.repeat(prel[r] * scale, D)
        wq.append(np.vstack([qw[d] * pr[None, :], (qb[d] * pr)[None, :]]
                            ).astype(np.float16))
    return wkv, wq, g('alw').astype(np.float16), g('alb'), g('ew'), g('eb')


def _prep_edges(ei, ea, et_min=0):
    src = np.asarray(ei[0]).astype(np.int64)
    dst = np.asarray(ei[1]).astype(np.int64)
    ea = np.asarray(ea)[:, 0].astype(np.float32)
    order = np.argsort(dst, kind='stable')
    src_s, dst_s, ea_s = src[order], dst[order], ea[order]
    tile_of = dst_s // P
    counts = np.bincount(tile_of, minlength=NT_A)
    ET = max(int(counts.max()), et_min)
    ET = ((ET + P - 1) // P) * P
    starts = np.zeros(NT_A, np.int64)
    starts[1:] = np.cumsum(counts)[:-1]
    pos = np.arange(E) - starts[tile_of]
    psrc = np.zeros((NT_A, ET), np.int64)
    pslot = np.full((NT_A, ET), 255, np.int64)
    pea = np.zeros((NT_A, ET), np.float32)
    psrc[tile_of, pos] = src_s
    pslot[tile_of, pos] = dst_s - tile_of * P
    pea[tile_of, pos] = ea_s
    return ET, psrc, pslot, pea


def _wrap_idx(idx):
    T, ET = idx.shape
    w = idx.reshape(T, ET // 16, 16).transpose(2, 0, 1).astype(np.int16)
    return np.ascontiguousarray(np.tile(w, (8, 1, 1)))   # [128, T, ET//16]


def _edge_host_arrays(psrc, pslot, pea, ET, core):
    NE = ET // P
    t0 = core * TPC
    src = psrc[t0:t0 + TPC]
    slot = pslot[t0:t0 + TPC]
    ea = pea[t0:t0 + TPC]
    gkv = _wrap_idx(src)
    qloc = np.where(slot < P, np.arange(TPC)[:, None] * P + np.minimum(slot, P - 1), 0)
    gq = _wrap_idx(qloc)
    sl = slot.reshape(TPC, NE, P).transpose(0, 2, 1)
    ws = (sl[:, :, :, None] == np.arange(P)[None, None, None, :])
    ws = np.ascontiguousarray(ws.astype(np.int8).reshape(TPC, P, NE * P))
    eat = np.ascontiguousarray(
        ea.reshape(TPC, NE, P).transpose(2, 0, 1).astype(np.float16))
    return gkv, gq, ws, eat


# ============================ device program ============================
def build_program(ET, stages=99, debug=False):
    NE = ET // P
    nc = bacc.Bacc("TRN2", target_bir_lowering=False, debug=False,
                   enable_asserts=False, num_devices=NCORES)
    din = lambda n, s, d: nc.dram_tensor(n, s, d, kind="ExternalInput").ap()
    dout = lambda n, s, d: nc.dram_tensor(n, s, d, kind="ExternalOutput").ap()
    dint = lambda n, s, d, **kw: nc.dram_tensor(n, s, d, **kw).ap()

    xaT = din("xaT", [F1, N], f16)
    xuT = din("xuT", [F1, N], f16)
    xT_sl = [din(f"x{t}T_sl", [F1, SLICE], f16) for t in (0, 1)]
    wkv1 = [din(f"wkv1_{r}", [F1 + 1, KV1], f16) for r in (0, 1)]
    wq1 = [din(f"wq1_{r}", [F1 + 1, HD1], f16) for r in (0, 1)]
    wkv2 = [din(f"wkv2_{r}", [F2 + 1, KV2], f16) for r in (0, 1)]
    wq2 = [din(f"wq2_{r}", [F2 + 1, HD2], f16) for r in (0, 1)]
    alw1 = din("alw1", [2, HD1, HD1], f16)
    alb1 = din("alb1", [2, HD1, 1], f32)
    alw2 = din("alw2", [2, HD2, HD2], f16)
    alb2 = din("alb2", [2, HD2, 1], f32)
    lin1w = din("lin1w", [2, 2 * F2 + 1, LIN], f16)   # bias row at 320
    lin2wa = din("lin2wa", [LIN, FA], f16)
    lin2wu = din("lin2wu", [LIN, FU], f16)
    lin2ba = din("lin2ba", [FA, 1], f32)
    lin2bu = din("lin2bu", [FU, 1], f32)
    bng = din("bng", [2, LIN, 1], f32)
    bnb = din("bnb", [2, LIN, 1], f32)
    ew_eb = din("ew_eb", [P, 2, 2, 2, H], f32)
    nphant = din("nphant", [P, 1], f32)
    gkv = [din(f"gkv_{r}", [P, TPC, ET // 16], i16) for r in (0, 1)]
    gq = [din(f"gq_{r}", [P, TPC, ET // 16], i16) for r in (0, 1)]
    wsm = [din(f"ws_{r}", [TPC, P, NE * P], i8) for r in (0, 1)]
    eat_d = [din(f"eat_{r}", [P, TPC, NE], f16) for r in (0, 1)]

    a_out = dout("a_out", [SLICE, FA], f32)
    u_out = dout("u_out", [SLICE, FU], f32)

    dkind = dict(kind="ExternalOutput") if debug else {}
    kvtab1 = [dint(f"kvtab1_{r}", [N, KVP1], f16, **dkind) for r in (0, 1)]
    qtab1 = [dint(f"qtab1_{r}", [SLICE, QP1], f16, **dkind) for r in (0, 1)]
    kvtab2 = [dint(f"kvtab2_{r}", [NTOT, KVP2], f16, **dkind) for r in (0, 1)]
    qtab2 = [dint(f"qtab2_{r}", [SLICE, QP2], f16, **dkind) for r in (0, 1)]
    a1T_loc = [dint(f"a1T_loc_{t}", [F2 + 1, SLICE], f16) for t in (0, 1)]
    a1T_dbg = ([dint(f"a1T_dbg_{t}", [F2 + 1, SLICE], f16, kind="ExternalOutput")
                for t in (0, 1)] if debug else None)
    a1T_full = [dint(f"a1T_full_{t}", [NCORES * (F2 + 1), SLICE], f16,
                     addr_space="Shared") for t in (0, 1)]
    stats_d = [dint(f"stats_d{t}", [4, P, 2], f32) for t in (0, 1)]
    h2T_dbg = (dint("h2T_dbg", [2, 2 * F2 + 1, SLICE], f16, kind="ExternalOutput")
               if debug else None)
    zT_dbg = (dint("zT_dbg", [2, 4, P, SLICE], f16, kind="ExternalOutput")
              if debug else None)
    st_dbg = (dint("st_dbg", [2, 4, P, 2], f32, kind="ExternalOutput")
              if debug else None)
    ot_dbg = (dint("ot_dbg", [2, TPC, P, FU], f32, kind="ExternalOutput")
              if debug else None)
    scsh_dbg = (dint("scsh_dbg", [2, P, 8], f32, kind="ExternalOutput")
                if debug else None)
    wsc_dbg = (dint("wsc_dbg", [2, 4, P, P], f16, kind="ExternalOutput")
               if debug else None)
    bps_dbg = (dint("bps_dbg", [2, 2, P, 1], f32, kind="ExternalOutput")
               if debug else None)
    stats_g = [dint(f"stats_g{t}", [4, P, 2], f32, addr_space="Shared")
               for t in (0, 1)]

    with tile.TileContext(nc) as tc, ExitStack() as _ctx:
        const = _ctx.enter_context(tc.tile_pool(name="const", bufs=1))
        sbR = _ctx.enter_context(tc.tile_pool(name="sbR", bufs=1))
        sbA = _ctx.enter_context(tc.tile_pool(name="sbA", bufs=3))
        sbE = _ctx.enter_context(tc.tile_pool(name="sbE", bufs=1))
        ps = _ctx.enter_context(tc.tile_pool(name="ps", bufs=2, space="PSUM"))
        psE = _ctx.enter_context(tc.tile_pool(name="psE", bufs=2, space="PSUM"))

        ident16 = const.tile([P, P], f16)
        make_identity(nc, ident16[:])
        ident32 = const.tile([P, P], f32)
        make_identity(nc, ident32[:])
        ones16 = const.tile([1, LIN], f16)
        nc.gpsimd.memset(ones16[:], 1.0)
        zeros32 = const.tile([P, LIN], f32)
        nc.gpsimd.memset(zeros32[:], 0.0)
        ewb = const.tile([P, 2, 2, 2, H], f32)
        nc.sync.dma_start(ewb[:], ew_eb[:])
        nph = const.tile([P, 1], f32)
        nc.sync.dma_start(nph[:], nphant[:])
        eps5 = const.tile([P, 1], f32)
        nc.gpsimd.memset(eps5[:], 1e-5)
        eps12 = const.tile([P, 1], f32)
        nc.gpsimd.memset(eps12[:], 1e-12)

        # ---------- dense projection, orientation A ----------
        def dense_A(xap, w_ap, F, CO, tab, n_rows):
            n_tiles = (n_rows + P - 1) // P
            for nt in range(n_tiles):
                nn = min(P, n_rows - nt * P)
                lhs = sbA.tile([P, P], f16, name="dx", tag="dx")
                nc.sync.dma_start(lhs[:, :nn], xap[:, nt * P:nt * P + nn])
                for co in range(0, CO, LIN):
                    cw = min(LIN, CO - co)
                    pt = ps.tile([P, LIN], f32, name="dp", tag="mm")
                    wt = sbA.tile([P, LIN], f16, name="dw", tag="dw", bufs=2)
                    nc.sync.dma_start(wt[:, :cw], w_ap[0:F, co:co + cw])
                    nc.tensor.matmul(pt[:nn, :cw], lhsT=lhs[:, :nn], rhs=wt[:, :cw],
                                     start=True, stop=False)
                    bt = sbA.tile([1, LIN], f16, name="db", tag="db")
                    nc.sync.dma_start(bt[:1, :cw], w_ap[F:F + 1, co:co + cw])
                    nc.tensor.matmul(pt[:nn, :cw], lhsT=ones16[:1, :nn],
                                     rhs=bt[:1, :cw], start=False, stop=True)
                    ev = sbA.tile([P, LIN], f16, name="de", tag="de", bufs=2)
                    nc.scalar.copy(ev[:nn, :cw], pt[:nn, :cw])
                    nc.sync.dma_start(tab[nt * P:nt * P + nn, co:co + cw],
                                      ev[:nn, :cw])

        if stages >= 1:
            dense_A(xaT, wkv1[0], F1, KV1, kvtab1[0], N)
            dense_A(xuT, wkv1[1], F1, KV1, kvtab1[1], N)
            dense_A(xT_sl[1], wq1[0], F1, HD1, qtab1[0], SLICE)
            dense_A(xT_sl[0], wq1[1], F1, HD1, qtab1[1], SLICE)

        # ---------- edge phase ----------
        ea_res = [sbR.tile([P, TPC, NE], f16, name=f"ear{r}", tag=f"ear{r}")
                  for r in (0, 1)]
        for r in (0, 1):
            nc.sync.dma_start(ea_res[r][:], eat_d[r][:])
        agg_d = [dint(f"agg_d{r}", [P, TPC, HD2], f16, **dkind) for r in (0, 1)]

        def edge_phase(l, r, kvtab, qtab, KVP, QPp, HD, D):
            li = l - 1
            for t in range(TPC):
                ikt = sbA.tile([P, ET // 16], i16, name="ikt", tag="ikt", bufs=3)
                nc.sync.dma_start(ikt[:], gkv[r][:, t, :])
                iqt = sbA.tile([P, ET // 16], i16, name="iqt", tag="iqt", bufs=3)
                nc.sync.dma_start(iqt[:], gq[r][:, t, :])
                kvg = sbE.tile([P, NE, KVP], f16, name="kvg", tag="kvg", bufs=2)
                nc.gpsimd.dma_gather(
                    out_ap=kvg[:], in_ap=kvtab[:], idxs_ap=ikt[:],
                    num_idxs=ET, num_idxs_reg=ET, elem_size=KVP, single_packet=False)
                qg = sbE.tile([P, NE, QPp], f16, name="qg", tag="qg", bufs=1)
                nc.gpsimd.dma_gather(
                    out_ap=qg[:], in_ap=qtab[:], idxs_ap=iqt[:],
                    num_idxs=ET, num_idxs_reg=ET, elem_size=QPp, single_packet=False)
                wst = sbE.tile([P, NE * P], f16, name="wst", tag="wst", bufs=1)
                nc.gpsimd.dma_start(out=wst[:], in_=wsm[r][t])
                # t = kr * q  (in place into qg)
                nc.vector.tensor_tensor(out=qg[:, :, :HD], in0=kvg[:, :, :HD],
                                        in1=qg[:, :, :HD], op=OP.mult)
                lg = sbA.tile([P, NE, H], f32, name="lg", tag="lg")
                nc.vector.tensor_reduce(
                    out=lg[:], in_=qg[:, :, :HD].rearrange("p n (h d) -> p n h d", h=H),
                    axis=AX.X, op=OP.add)
                cg = sbA.tile([P, NE, H], f32, name="cg", tag="cg")
                ewsl = ewb[:, li, r, 0, :]
                ebsl = ewb[:, li, r, 1, :]
                ew_b = bass.AP(ewsl.tensor, ewsl.offset,
                               [list(ewsl.ap[0]), [0, NE], [1, H]])
                eb_b = bass.AP(ebsl.tensor, ebsl.offset,
                               [list(ebsl.ap[0]), [0, NE], [1, H]])
                nc.vector.tensor_tensor(
                    out=cg[:], in0=ea_res[r][:, t, :].to_broadcast([P, NE, H]),
                    in1=ew_b, op=OP.mult)
                nc.vector.tensor_tensor(out=cg[:], in0=cg[:], in1=eb_b, op=OP.add)
                nc.vector.tensor_tensor(out=lg[:], in0=lg[:], in1=cg[:], op=OP.add)
                # e broadcast into the (consumed) kr slots of kvg
                lgs = lg[:]
                nc.scalar.activation(
                    kvg[:, :, :HD].rearrange("p n (h d) -> p n h d", h=H),
                    bass.AP(lgs.tensor, lgs.offset,
                            [list(lgs.ap[0]), [H, NE], [1, H], [0, D]]),
                    AF.Exp)
                # e compact: L1 -> pad cols of kvg; L2 -> separate tile
                if KVP >= 2 * HD + H:
                    nc.scalar.activation(kvg[:, :, 2 * HD:2 * HD + H], lg[:], AF.Exp)
                    ec = kvg[:, :, 2 * HD:2 * HD + H]
                else:
                    ect = sbE.tile([P, NE, H], f16, name="ect", tag="ect", bufs=2)
                    nc.scalar.activation(ect[:], lg[:], AF.Exp)
                    ec = ect[:]
                # msg = vr * e  (in place into vr slots)
                nc.vector.tensor_tensor(out=kvg[:, :, HD:2 * HD],
                                        in0=kvg[:, :, HD:2 * HD],
                                        in1=kvg[:, :, :HD], op=OP.mult)
                ap_t = psE.tile([P, HD + H], f32, name="aggp", tag="aggp")
                if KVP >= 2 * HD + H:
                    for j in range(NE):
                        nc.tensor.matmul(
                            ap_t[:], lhsT=wst[:, j * P:(j + 1) * P],
                            rhs=kvg[:, j, HD:2 * HD + H],
                            start=(j == 0), stop=(j == NE - 1))
                    s_ap = ap_t[:, HD:HD + H]
                else:
                    ap_s = psE.tile([P, H], f32, name="aggs", tag="aggs")
                    for j in range(NE):
                        nc.tensor.matmul(
                            ap_t[:, :HD], lhsT=wst[:, j * P:(j + 1) * P],
                            rhs=kvg[:, j, HD:2 * HD],
                            start=(j == 0), stop=(j == NE - 1))
                    for j in range(NE):
                        nc.tensor.matmul(
                            ap_s[:], lhsT=wst[:, j * P:(j + 1) * P],
                            rhs=ec[:, j, :],
                            start=(j == 0), stop=(j == NE - 1))
                    s_ap = ap_s[:]
                sinv = sbA.tile([P, H], f32, name="sinv", tag="sinv")
                nc.vector.tensor_scalar_add(sinv[:], in0=s_ap,
                                            scalar1=1e-16)
                nc.vector.reciprocal(sinv[:], sinv[:])
                af = sbA.tile([P, HD2], f16, name="aggf", tag="aggf", bufs=3)
                nc.vector.tensor_tensor(
                    out=af[:, :HD].rearrange("p (h d) -> p h d", h=H),
                    in0=ap_t[:, :HD].rearrange("p (h d) -> p h d", h=H),
                    in1=sinv[:].to_broadcast([P, H, D]), op=OP.mult)
                nc.sync.dma_start(agg_d[r][:, t, :HD], af[:, :HD])

        if stages >= 2:
            edge_phase(1, 0, kvtab1[0], qtab1[0], KVP1, QP1, HD1, D1)
            edge_phase(1, 1, kvtab1[1], qtab1[1], KVP1, QP1, HD1, D1)

        # ---------- node phase ----------
        def node_phase(l, alw_ap, alb_ap, HD, out_chunks):
            nkc = (HD + P - 1) // P
            for ty in (0, 1):
                aggT = [sbA.tile([P, SLICE], f16, name=f"aggT{k}", tag=f"aggT{k}",
                                 bufs=1) for k in range(nkc)]
                for t in range(TPC):
                    agl = sbA.tile([P, HD2], f16, name="agl", tag="agl", bufs=2)
                    nc.sync.dma_start(agl[:, :HD], agg_d[1 - ty][:, t, :HD])
                    gel = sbA.tile([P, HD2], f16, name="gel", tag="gel", bufs=2)
                    nc.scalar.activation(gel[:, :HD], agl[:, :HD],
                                         AF.Gelu_apprx_tanh)
                    for k in range(nkc):
                        rows = min(P, HD - k * P)
                        pt = ps.tile([P, P], f16, name="trp", tag="trp")
                        nc.tensor.transpose(pt[:rows, :], gel[:, k * P:k * P + rows],
                                            ident16[:])
                        nc.vector.tensor_copy(aggT[k][:rows, t * P:(t + 1) * P],
                                              pt[:rows, :])
                alb_sb = sbA.tile([P, nkc], f32, name="albs", tag="albs")
                for k in range(nkc):
                    rows = min(P, HD - k * P)
                    nc.sync.dma_start(alb_sb[:rows, k:k + 1],
                                      alb_ap[ty, k * P:k * P + rows, :])
                for ci in range(nkc):
                    crows = min(P, HD - ci * P)
                    for nk in range(SLICE // LIN):
                        pt = ps.tile([P, LIN], f32, name="alwp", tag="mm")
                        for k in range(nkc):
                            rows = min(P, HD - k * P)
                            wt = sbA.tile([P, P], f16, name="alwW", tag="alwW")
                            nc.sync.dma_start(
                                wt[:rows, :crows],
                                alw_ap[ty, k * P:k * P + rows, ci * P:ci * P + crows])
                            nc.tensor.matmul(
                                pt[:crows, :], lhsT=wt[:rows, :crows],
                                rhs=aggT[k][:rows, nk * LIN:(nk + 1) * LIN],
                                start=(k == 0), stop=(k == nkc - 1))
                        ns = slice(nk * LIN, (nk + 1) * LIN)
                        tm = sbA.tile([P, LIN], f32, name="ntm", tag="ntm", bufs=2)
                        nc.vector.scalar_tensor_tensor(
                            out=tm[:crows, :], in0=pt[:crows, :],
                            scalar=alb_sb[:crows, ci:ci + 1],
                            in1=zeros32[:crows, :], op0=OP.add, op1=OP.min)
                        te = sbA.tile([P, LIN], f32, name="nte", tag="nte", bufs=2)
                        nc.scalar.activation(te[:crows, :], tm[:crows, :], AF.Exp)
                        tp = sbA.tile([P, LIN], f32, name="ntp", tag="ntp", bufs=2)
                        nc.vector.scalar_tensor_tensor(
                            out=tp[:crows, :], in0=pt[:crows, :],
                            scalar=alb_sb[:crows, ci:ci + 1],
                            in1=zeros32[:crows, :], op0=OP.add, op1=OP.max)
                        nc.vector.scalar_tensor_tensor(
                            out=out_chunks[ty][ci][:crows, ns], in0=te[:crows, :],
                            scalar=-1.0, in1=tp[:crows, :], op0=OP.add, op1=OP.add)

        a1T_sb = {}
        for ty in (0, 1):
            c0 = sbR.tile([P, SLICE], f16, name=f"a1T{ty}0", tag=f"tpA{ty}0")
            c1 = sbR.tile([33, SLICE], f16, name=f"a1T{ty}1", tag=f"tpA{ty}1")
            nc.gpsimd.memset(c1[32:33, :], 1.0)
            a1T_sb[ty] = [c0, c1]
        if stages >= 3:
            node_phase(1, alw1, alb1, HD1, {0: a1T_sb[0], 1: a1T_sb[1]})

        if stages >= 4:
            for ty in (0, 1):
                nc.sync.dma_start(a1T_loc[ty][0:P, :], a1T_sb[ty][0][:])
                nc.sync.dma_start(a1T_loc[ty][P:P + 33, :], a1T_sb[ty][1][:])
                if debug:
                    nc.sync.dma_start(a1T_dbg[ty][0:P, :], a1T_sb[ty][0][:])
                    nc.sync.dma_start(a1T_dbg[ty][P:P + 33, :], a1T_sb[ty][1][:])
                nc.gpsimd.collective_compute(
                    "AllGather", OP.bypass, replica_groups=[list(range(NCORES))],
                    ins=[a1T_loc[ty][:]], outs=[a1T_full[ty][:]])

        # ---------- L2 dense ----------
        def kv2_phase(src_ty, w_ap, tab):
            af = a1T_full[src_ty]
            for nt in range(NT_A):
                blk, off = (nt * P) // SLICE, (nt * P) % SLICE
                base = blk * (F2 + 1)
                lhs0 = sbA.tile([P, P], f16, name="d2x0", tag="d2x0")
                nc.sync.dma_start(lhs0[:], af[base:base + P, off:off + P])
                lhs1 = sbA.tile([33, P], f16, name="d2x1", tag="d2x1")
                nc.sync.dma_start(lhs1[:], af[base + P:base + P + 33, off:off + P])
                for co in range(0, KV2, LIN):
                    cw = min(LIN, KV2 - co)
                    pt = ps.tile([P, LIN], f32, name="d2p", tag="mm")
                    wt0 = sbA.tile([P, LIN], f16, name="d2w0", tag="d2w0", bufs=2)
                    nc.sync.dma_start(wt0[:, :cw], w_ap[0:P, co:co + cw])
                    nc.tensor.matmul(pt[:, :cw], lhsT=lhs0[:], rhs=wt0[:, :cw],
                                     start=True, stop=False)
                    wt1 = sbA.tile([33, LIN], f16, name="d2w1", tag="d2w1")
                    nc.sync.dma_start(wt1[:, :cw], w_ap[P:P + 33, co:co + cw])
                    nc.tensor.matmul(pt[:, :cw], lhsT=lhs1[:], rhs=wt1[:, :cw],
                                     start=False, stop=True)
                    ev = sbA.tile([P, LIN], f16, name="d2e", tag="d2e", bufs=2)
                    nc.scalar.copy(ev[:, :cw], pt[:, :cw])
                    nc.sync.dma_start(tab[nt * P:(nt + 1) * P, co:co + cw],
                                      ev[:, :cw])

        if stages >= 5:
            kv2_phase(0, wkv2[0], kvtab2[0])
            kv2_phase(1, wkv2[1], kvtab2[1])

        def q2_phase(dst_ty, w_ap, tab):
            for st in range(TPC):
                pt = ps.tile([P, HD2], f32, name="q2p", tag="mm")
                wt0 = sbA.tile([P, HD2], f16, name="q2w0", tag="q2w0")
                nc.sync.dma_start(wt0[:], w_ap[0:P, :])
                nc.tensor.matmul(pt[:], lhsT=a1T_sb[dst_ty][0][:, st * P:(st + 1) * P],
                                 rhs=wt0[:], start=True, stop=False)
                wt1 = sbA.tile([33, HD2], f16, name="q2w1", tag="q2w1")
                nc.sync.dma_start(wt1[:], w_ap[P:P + 33, :])
                nc.tensor.matmul(pt[:], lhsT=a1T_sb[dst_ty][1][:, st * P:(st + 1) * P],
                                 rhs=wt1[:], start=False, stop=True)
                ev = sbA.tile([P, HD2], f16, name="q2e", tag="q2e")
                nc.scalar.copy(ev[:], pt[:])
                nc.sync.dma_start(tab[st * P:(st + 1) * P, :HD2], ev[:])

        if stages >= 5:
            q2_phase(1, wq2[0], qtab2[0])
            q2_phase(0, wq2[1], qtab2[1])

        if stages >= 6:
            edge_phase(2, 0, kvtab2[0], qtab2[0], KVP2, QP2, HD2, D2)
            edge_phase(2, 1, kvtab2[1], qtab2[1], KVP2, QP2, HD2, D2)

        h2T_sb = {}
        for ty in (0, 1):
            c0 = sbR.tile([P, SLICE], f16, name=f"h2T{ty}0", tag=f"tpA{ty}0")
            c1 = sbR.tile([P, SLICE], f16, name=f"h2T{ty}1", tag=f"tpA{ty}1")
            c2 = sbR.tile([65, SLICE], f16, name=f"h2T{ty}2", tag=f"tpB{ty}2")
            nc.gpsimd.memset(c2[64:65, :], 1.0)
            h2T_sb[ty] = [c0, c1, c2]
        if stages >= 7:
            node_phase(2, alw2, alb2, HD2, {0: h2T_sb[0], 1: h2T_sb[1]})
            if debug:
                for ty in (0, 1):
                    nc.sync.dma_start(h2T_dbg[ty, 0:P, :], h2T_sb[ty][0][:])
                    nc.sync.dma_start(h2T_dbg[ty, P:2 * P, :], h2T_sb[ty][1][:])
                    nc.sync.dma_start(h2T_dbg[ty, 2 * P:2 * P + 65, :],
                                      h2T_sb[ty][2][:])

        # ---------- final (per type) ----------
        kch = [(0, 0, P, 0), (1, 0, P, P), (2, 0, 65, 2 * P)]
        for ty, (l2w, l2b, CO, od) in ([] if stages < 8 else list(enumerate(
                ((lin2wa, lin2ba, FA, a_out), (lin2wu, lin2bu, FU, u_out))))):
            zT = [sbA.tile([P, SLICE], f16, name=f"zT{k}",
                           tag=(f"aggT{k}" if k < 3 else f"zT{k}"), bufs=1)
                  for k in range(4)]
            for ci in range(4):
                for nk in range(SLICE // LIN):
                    pt = ps.tile([P, LIN], f32, name="l1p", tag="mm")
                    for ki, (hi, r0, rows, wr) in enumerate(kch):
                        wt = sbA.tile([P, P], f16, name="l1w", tag="l1w")
                        nc.sync.dma_start(
                            wt[:rows, :], lin1w[ty, wr:wr + rows, ci * P:(ci + 1) * P])
                        nc.tensor.matmul(
                            pt[:], lhsT=wt[:rows, :],
                            rhs=h2T_sb[ty][hi][r0:r0 + rows, nk * LIN:(nk + 1) * LIN],
                            start=(ki == 0), stop=(ki == 2))
                    ns = slice(nk * LIN, (nk + 1) * LIN)
                    tm = sbA.tile([P, LIN], f32, name="ntm", tag="ntm", bufs=2)
                    nc.vector.tensor_scalar_min(tm[:], in0=pt[:], scalar1=0.0)
                    te = sbA.tile([P, LIN], f32, name="nte", tag="nte", bufs=2)
                    nc.scalar.activation(te[:], tm[:], AF.Exp)
                    tp = sbA.tile([P, LIN], f32, name="ntp", tag="ntp", bufs=2)
                    nc.vector.tensor_scalar_max(tp[:], in0=pt[:], scalar1=0.0)
                    nc.vector.scalar_tensor_tensor(
                        out=zT[ci][:, ns], in0=te[:], scalar=-1.0,
                        in1=tp[:], op0=OP.add, op1=OP.add)
            st_t = sbA.tile([P, 2, 4], f32, name="stt", tag="stt")
            for ci in range(4):
                z = zT[ci]
                nc.vector.reduce_sum(st_t[:, 0, ci:ci + 1], z[:], axis=AX.X)
                sq = sbA.tile([P, SLICE], f16, name="fsq", tag="fsq", bufs=1)
                nc.scalar.activation(sq[:], z[:], AF.Square,
                                     accum_out=st_t[:, 1, ci:ci + 1])
                pc = sbA.tile([P, 2], f32, name="fpc", tag="fpc")
                nc.vector.tensor_copy(pc[:, 0:1], z[:, SLICE - 1:SLICE])
                nc.vector.tensor_tensor(out=pc[:, 1:2], in0=pc[:, 0:1],
                                        in1=pc[:, 0:1], op=OP.mult)
                nc.vector.scalar_tensor_tensor(
                    out=st_t[:, 0, ci:ci + 1], in0=pc[:, 0:1], scalar=nph[:],
                    in1=st_t[:, 0, ci:ci + 1], op0=OP.mult, op1=OP.add)
                nc.vector.scalar_tensor_tensor(
                    out=st_t[:, 1, ci:ci + 1], in0=pc[:, 1:2], scalar=nph[:],
                    in1=st_t[:, 1, ci:ci + 1], op0=OP.mult, op1=OP.add)
                nc.sync.dma_start(stats_d[ty][ci, :, :], st_t[:, :, ci])
            if debug:
                for k in range(4):
                    nc.sync.dma_start(zT_dbg[ty, k, :, :], zT[k][:])
            nc.gpsimd.collective_compute(
                "AllReduce", OP.add, replica_groups=[list(range(NCORES))],
                ins=[stats_d[ty][:]], outs=[stats_g[ty][:]])
            if debug:
                sgd = sbA.tile([P, 4, 2], f32, name="sgd", tag="sgd")
                for k in range(4):
                    nc.sync.dma_start(sgd[:, k, :], stats_g[ty][k, :, :])
                nc.sync.dma_start(st_dbg[ty][:].rearrange("c p s -> p c s"), sgd[:])
            sc_t = sbA.tile([P, 4], f32, name="fsc", tag="fsc")
            sh_t = sbA.tile([P, 4], f16, name="fsh", tag="fsh")
            for ci in range(4):
                sg = sbA.tile([P, 2], f32, name="fsg", tag="fsg")
                nc.sync.dma_start(sg[:], stats_g[ty][ci, :, :])
                mu = sbA.tile([P, 4], f32, name="fmu", tag="fmu")
                nc.vector.tensor_scalar_mul(mu[:, 0:1], in0=sg[:, 0:1], scalar1=1.0 / N)
                nc.vector.tensor_scalar_mul(mu[:, 1:2], in0=sg[:, 1:2], scalar1=1.0 / N)
                nc.vector.tensor_tensor(out=mu[:, 2:3], in0=mu[:, 0:1],
                                        in1=mu[:, 0:1], op=OP.mult)
                nc.vector.tensor_tensor(out=mu[:, 3:4], in0=mu[:, 1:2],
                                        in1=mu[:, 2:3], op=OP.subtract)
                vs = sbA.tile([P, 1], f32, name="fvs", tag="fvs")
                nc.scalar.activation(vs[:], mu[:, 3:4], AF.Sqrt, bias=eps5[:])
                nc.vector.reciprocal(vs[:], vs[:])
                gt = sbA.tile([P, 2], f32, name="fgt", tag="fgt")
                nc.sync.dma_start(gt[:, 0:1], bng[ty, ci * P:(ci + 1) * P, :])
                nc.sync.dma_start(gt[:, 1:2], bnb[ty, ci * P:(ci + 1) * P, :])
                nc.vector.tensor_tensor(out=sc_t[:, ci:ci + 1], in0=gt[:, 0:1],
                                        in1=vs[:], op=OP.mult)
                tmp = sbA.tile([P, 1], f32, name="ftmp", tag="ftmp")
                nc.vector.tensor_tensor(out=tmp[:], in0=mu[:, 0:1],
                                        in1=sc_t[:, ci:ci + 1], op=OP.mult)
                nc.vector.scalar_tensor_tensor(
                    out=sh_t[:, ci:ci + 1], in0=tmp[:], scalar=-1.0,
                    in1=gt[:, 1:2], op0=OP.mult, op1=OP.add)
            if debug:
                nc.sync.dma_start(scsh_dbg[ty, :, 0:4], sc_t[:])
                shf = sbA.tile([P, 4], f32, name="shf", tag="shf")
                nc.vector.tensor_copy(shf[:], sh_t[:])
                nc.sync.dma_start(scsh_dbg[ty, :, 4:8], shf[:])
            nco = (CO + P - 1) // P
            wsc_all, bps_all = [], []
            for co in range(nco):
                crows = min(P, CO - co * P)
                wsc = [sbA.tile([P, P], f16, name=f"l2w{k}_{co}", tag=f"l2w{k}_{co}", bufs=1)
                       for k in range(4)]
                bp = ps.tile([P, 1], f32, name="l2bp", tag="mm")
                for k in range(4):
                    wt = sbA.tile([P, P], f16, name="l2wl", tag="l2wl")
                    nc.sync.dma_start(wt[:, :crows],
                                      l2w[k * P:(k + 1) * P, co * P:co * P + crows])
                    nc.vector.tensor_scalar(
                        out=wsc[k][:, :crows], in0=wt[:, :crows],
                        scalar1=sc_t[:, k:k + 1], scalar2=None, op0=OP.mult)
                    nc.tensor.matmul(bp[:crows, :], lhsT=wt[:, :crows],
                                     rhs=sh_t[:, k:k + 1], start=(k == 0),
                                     stop=(k == 3))
                bps = sbA.tile([P, 1], f32, name=f"l2bs{co}", tag=f"l2bs{co}")
                bt = sbA.tile([P, 1], f32, name="l2bt", tag="l2bt")
                nc.sync.dma_start(bt[:crows, :], l2b[co * P:co * P + crows, :])
                nc.vector.tensor_tensor(out=bps[:crows, :], in0=bp[:crows, :],
                                        in1=bt[:crows, :], op=OP.add)
                if debug:
                    if co == 0:
                        for k in range(4):
                            nc.sync.dma_start(wsc_dbg[ty, k, :, :], wsc[k][:])
                    nc.sync.dma_start(bps_dbg[ty, co, :crows, :], bps[:crows, :])
                wsc_all.append(wsc)
                bps_all.append(bps)
            for nt in range(TPC):
                ot = sbA.tile([P, FU], f32, name="l2o", tag="l2o")
                for co in range(nco):
                    crows = min(P, CO - co * P)
                    pt = ps.tile([P, P], f32, name="l2p", tag="mm")
                    for k in range(4):
                        nc.tensor.matmul(
                            pt[:crows, :], lhsT=wsc_all[co][k][:, :crows],
                            rhs=zT[k][:, nt * P:(nt + 1) * P],
                            start=(k == 0), stop=(k == 3))
                    ev = sbA.tile([P, P], f32, name="l2e", tag="l2e")
                    nc.vector.tensor_scalar(out=ev[:crows, :], in0=pt[:crows, :],
                                            scalar1=bps_all[co][:crows, :],
                                            scalar2=None, op0=OP.add)
                    pt2 = ps.tile([P, P], f32, name="l2t", tag="trp")
                    nc.tensor.transpose(pt2[:, :crows], ev[:crows, :],
                                        ident32[:crows, :crows])
                    nc.vector.tensor_copy(ot[:, co * P:co * P + crows], pt2[:, :crows])
                if debug:
                    nc.sync.dma_start(ot_dbg[ty, nt, :, :CO], ot[:, :CO])
                if ty == 0:
                    s2 = sbA.tile([P, FA], f32, name="fs2", tag="fs2")
                    ssq = sbA.tile([P, 1], f32, name="fssq", tag="fssq")
                    nc.scalar.activation(s2[:], ot[:, :FA], AF.Square,
                                         accum_out=ssq[:])
                    ssq2 = sbA.tile([P, 1], f32, name="fssq2", tag="fssq2")
                    nc.scalar.activation(ssq2[:], ssq[:], AF.Sqrt, bias=eps12[:])
                    nc.vector.reciprocal(ssq2[:], ssq2[:])
                    fo = sbA.tile([P, FA], f32, name="fo0", tag="fo0")
                    nc.vector.tensor_scalar(out=fo[:], in0=ot[:, :FA],
                                            scalar1=ssq2[:], scalar2=None, op0=OP.mult)
                    nc.sync.dma_start(od[nt * P:(nt + 1) * P, :], fo[:])
                else:
                    mx = sbA.tile([P, 1], f32, name="fmx", tag="fmx")
                    nc.vector.tensor_reduce(mx[:], ot[:], axis=AX.X, op=OP.max)
                    nc.vector.tensor_scalar_mul(mx[:], in0=mx[:], scalar1=-1.0)
                    se = sbA.tile([P, 1], f32, name="fse", tag="fse")
                    eo2 = sbA.tile([P, FU], f32, name="feo", tag="feo")
                    nc.scalar.activation(eo2[:], ot[:], AF.Exp, bias=mx[:],
                                         accum_out=se[:])
                    nc.vector.reciprocal(se[:], se[:])
                    fo = sbA.tile([P, FU], f32, name="fo1", tag="fo1")
                    nc.vector.tensor_scalar(out=fo[:], in0=eo2[:], scalar1=se[:],
                                            scalar2=None, op0=OP.mult)
                    nc.sync.dma_start(od[nt * P:(nt + 1) * P, :], fo[:])

    nc.compile()
    return nc


# ============================ host orchestration ============================
def _build_in_maps(inp):
    wkv1, wq1, alw1, alb1, ew1, eb1 = _fold_weights(inp, 'l1_', D1)
    wkv2, wq2, alw2, alb2, ew2, eb2 = _fold_weights(inp, 'l2_', D2)
    ET, psrc0, pslot0, pea0 = _prep_edges(inp['ei_au'], inp['ea_au'])
    ET2, psrc1, pslot1, pea1 = _prep_edges(inp['ei_ua'], inp['ea_ua'], et_min=ET)
    if ET2 > ET:
        ET, psrc0, pslot0, pea0 = _prep_edges(inp['ei_au'], inp['ea_au'], et_min=ET2)
    ET = max(ET, ET2)

    xaT = np.ascontiguousarray(np.asarray(inp['x_ant'], np.float32).T
                               ).astype(np.float16)
    xuT = np.ascontiguousarray(np.asarray(inp['x_user'], np.float32).T
                               ).astype(np.float16)
    xaT_pad = np.zeros((F1, NTOT), np.float16)
    xaT_pad[:, :N] = xaT
    xuT_pad = np.zeros((F1, NTOT), np.float16)
    xuT_pad[:, :N] = xuT

    ew_eb = np.zeros((P, 2, 2, 2, H), np.float32)
    for li, (ews, ebs) in enumerate(((ew1, eb1), (ew2, eb2))):
        for r in (0, 1):
            ew_eb[:, li, r, 0, :] = ews[r][0][None, :]
            ew_eb[:, li, r, 1, :] = ebs[r][None, :]

    lin1w = np.asarray(inp['lin1_w'], np.float32)
    lin1b = np.asarray(inp['lin1_b'], np.float32)
    lin1w_aug = np.concatenate([lin1w, lin1b[:, None, :]], axis=1).astype(np.float16)

    shared = {
        'xaT': xaT, 'xuT': xuT,
        'wkv1_0': wkv1[0], 'wkv1_1': wkv1[1], 'wq1_0': wq1[0], 'wq1_1': wq1[1],
        'wkv2_0': wkv2[0], 'wkv2_1': wkv2[1], 'wq2_0': wq2[0], 'wq2_1': wq2[1],
        'alw1': alw1, 'alb1': alb1[:, :, None].astype(np.float32),
        'alw2': alw2, 'alb2': alb2[:, :, None].astype(np.float32),
        'lin1w': lin1w_aug,
        'lin2wa': np.asarray(inp['lin2_wa'], np.float32).astype(np.float16),
        'lin2wu': np.asarray(inp['lin2_wu'], np.float32).astype(np.float16),
        'lin2ba': np.asarray(inp['lin2_ba'], np.float32)[:, None],
        'lin2bu': np.asarray(inp['lin2_bu'], np.float32)[:, None],
        'bng': np.asarray(inp['bn_g'], np.float32)[:, :, None],
        'bnb': np.asarray(inp['bn_b'], np.float32)[:, :, None],
        'ew_eb': ew_eb,
    }
    in_maps = []
    for c in range(NCORES):
        m = dict(shared)
        m['x0T_sl'] = np.ascontiguousarray(xaT_pad[:, c * SLICE:(c + 1) * SLICE])
        m['x1T_sl'] = np.ascontiguousarray(xuT_pad[:, c * SLICE:(c + 1) * SLICE])
        for r, (psrc, pslot, pea) in enumerate(((psrc0, pslot0, pea0),
                                                (psrc1, pslot1, pea1))):
            gkv_a, gq_a, ws_a, eat_a = _edge_host_arrays(psrc, pslot, pea, ET, c)
            m[f'gkv_{r}'], m[f'gq_{r}'] = gkv_a, gq_a
            m[f'ws_{r}'], m[f'eat_{r}'] = ws_a, eat_a
        nreal = max(0, min(SLICE, N - c * SLICE))
        m['nphant'] = np.full((P, 1), -(SLICE - nreal), np.float32)
        in_maps.append(m)
    return in_maps, ET


def kernel(**inputs):
    import os
    stages = int(os.environ.get('HETGAT_STAGES', '99'))
    debug = bool(int(os.environ.get('HETGAT_DEBUG', '0')))
    in_maps, ET = _build_in_maps(inputs)
    key = ('prog', ET, stages, debug)
    if key not in _cache:
        _cache[key] = build_program(ET, stages, debug)
    nc = _cache[key]
    res = run_bass_kernel_spmd(nc, in_maps, list(range(NCORES)))
    kernel._last = res
    a = np.concatenate([res.results[c]['a_out'] for c in range(NCORES)], 0)[:N]
    u = np.concatenate([res.results[c]['u_out'] for c in range(NCORES)], 0)[:N]
    return a, u
